# revision 1
# baseline (speedup 1.0000x reference)
"""Converse2D-Up (FFT deconvolution upsampler) as a Bass/Tile kernel for TRN2.

Math (validated against the jax reference to rel-l2 ~1.1e-4 == the
reference's own fp32 noise floor):

The whole pipeline before the final gelu is linear in x and channel-wise.
With xp = wrap-pad(x) (132x132), Y = FFT132(xp) = G @ x @ G^T where
G = F132 @ P (132x128, P = periodic pad selection).  The reference's
264-point spectral transfer function H (built from weight/bias only) is
Hermitian, so out = crop(real(IFFT264(H . tile(Y)))) decomposes into 4
polyphase outputs out_dd = real(IFFT132(Kdd_hat . Y)) with per-channel
precomputed spectra Kdd_hat; the crop leaves exactly 128 rows/cols per
phase.  Hermitian symmetry further means only columns v=0..66 of
Kdd_hat.Y are needed:
    T1[x,v] = sum_u Ai[x,u] (Kdd_hat.Y)[u,v]      (Ai = cropped iF132 rows)
    out[x,y] = sum_{v=0..66} w_v Re(T1[x,v] Ai[y,v]),  w = [1,2,...,2,1]
Everything maps onto fp32 PE matmuls with K<=132 contractions (split
128+4), a small pointwise complex multiply (DVE for the 128-row chunk,
GPSIMD for the 4-row chunk), and gelu+phase-interleave fused into the
ScalarE PSUM->SBUF eviction.

Sharding: 8 channels per core x 4 batch images (all per-(B,C)
independent); weight/bias-derived spectra are host-precomputed constants.
"""

import os

import numpy as np

import concourse.bass as bass
import concourse.mybir as mybir
import concourse.tile as tile
from concourse import bacc
from concourse.bass import ts
from concourse.bass_utils import run_bass_kernel_spmd

F32 = mybir.dt.float32
AF = mybir.ActivationFunctionType

SCALE = 2
PAD = 2
EPS = 1e-5
N0 = 128           # input spatial size
NP = N0 + 2 * PAD  # 132 padded
NU = NP * SCALE    # 264 upsampled
NV = NP // 2 + 1   # 67 unique spectral columns
B = 4
C = 64
NCORES = 8
CPC = C // NCORES  # 8 channels per core
NIMG = B * CPC     # 32 images per core

LAST_EXEC_NS = None  # set by kernel() when tracing is enabled


# --------------------------------------------------------------------------
# host-side constant precompute (weight/bias -> per-channel spectra)
# --------------------------------------------------------------------------

def _host_constants(weight, bias):
    w64 = np.asarray(weight, dtype=np.float64)
    b64 = np.asarray(bias, dtype=np.float64)

    # FB = p2o(weight): 264-point OTF of the rolled 3x3 PSF, per channel
    k_h, k_w = w64.shape[-2:]
    otf = np.zeros((C, NU, NU), dtype=np.complex128)
    otf[:, :k_h, :k_w] = w64[0]
    otf = np.roll(otf, (-(k_h // 2), -(k_w // 2)), axis=(-2, -1))
    FB = np.fft.fftn(otf, axes=(-2, -1))                      # (C,264,264)

    biaseps = 1.0 / (1.0 + np.exp(-(b64.reshape(C) - 9.0))) + EPS  # (C,)
    be = biaseps[:, None, None]

    u = np.arange(NU)
    Dr = 1 + np.exp(-2j * np.pi * u / NU)
    D = Dr[:, None] * Dr[None, :]                             # (264,264)

    Gh = np.conj(FB) + be * D[None]
    FBG = FB * Gh

    def quadmean(A):
        return 0.25 * (A[:, :NP, :NP] + A[:, NP:, :NP]
                       + A[:, :NP, NP:] + A[:, NP:, NP:])

    M1 = quadmean(FBG)
    invW = quadmean(np.abs(FB) ** 2)
    M2 = M1 / (invW + be)
    H = (Gh - np.conj(FB) * np.tile(M2, (1, SCALE, SCALE))) / be   # (C,264,264)

    hr = np.fft.ifft2(H, axes=(-2, -1)).real                  # H Hermitian
    # polyphase spectra: Kdd_hat[c,dx,dy] = FFT132(hr[c, dx::2, dy::2])
    kdd = np.empty((C, 2, 2, NP, NV), dtype=np.complex128)
    for dx in range(2):
        for dy in range(2):
            kh = np.fft.fft2(hr[:, dx::2, dy::2], axes=(-2, -1))
            kdd[:, dx, dy] = kh[:, :, :NV]

    # pack per channel: [u, plane(3), phase(4), v] planes = [Kr, Ki, Kr]
    kr = np.empty((C, NP, 4, NV), dtype=np.float32)
    ki = np.empty((C, NP, 4, NV), dtype=np.float32)
    for dx in range(2):
        for dy in range(2):
            p = dx * 2 + dy
            kr[:, :, p, :] = kdd[:, dx, dy].real.astype(np.float32)
            ki[:, :, p, :] = kdd[:, dx, dy].imag.astype(np.float32)
    kdd_packed = np.concatenate(
        [kr.reshape(C, NP, 4 * NV), ki.reshape(C, NP, 4 * NV),
         kr.reshape(C, NP, 4 * NV)], axis=2,
    )                                                          # (C,132,804)

    # forward matrix G = F132 @ P  (132x128 complex)
    P = np.zeros((NP, N0))
    for m in range(NP):
        P[m, (m - PAD) % N0] = 1.0
    F132 = np.exp(-2j * np.pi * np.outer(np.arange(NP), np.arange(NP)) / NP)
    G = F132 @ P

    gt = np.concatenate([G.real.T, G.imag.T], axis=1).astype(np.float32)   # (128,264)
    neg67 = (-G.imag.T[:, :NV]).astype(np.float32)                          # (128,67)

    # inverse matrix, rows i in [2,130) of iF132/132
    Ai = np.exp(2j * np.pi * np.outer(np.arange(2, 130), np.arange(NP)) / NP) / NP
    Cm, Sm = Ai.real, Ai.imag                                  # (128,132)
    CT, ST = Cm.T, Sm.T                                        # (132,128)
    cst = np.concatenate([CT, ST, -ST], axis=1).astype(np.float32)  # (132,384)

    w_v = np.ones(NV)
    w_v[1:NV - 1] = 2.0
    RC = (Cm[:, :NV] * w_v[None, :]).T.astype(np.float32)      # (67,128)
    RS = (-Sm[:, :NV] * w_v[None, :]).T.astype(np.float32)
    rcs = np.concatenate([RC, RS], axis=1).astype(np.float32)  # (67,256)

    return {
        "kdd_packed": kdd_packed.astype(np.float32),
        "gt": gt,
        "neg67": neg67,
        "cst_hi": np.ascontiguousarray(cst[:128]),
        "cst_lo": np.ascontiguousarray(cst[128:]),
        "rcs": rcs,
    }


# --------------------------------------------------------------------------
# device kernel
# --------------------------------------------------------------------------

def build_nc(n_chan=CPC, n_batch=B, gelu=True):
    act_fn = AF.Gelu if gelu else AF.Copy
    n_img = n_chan * n_batch
    nc = bacc.Bacc("TRN2", target_bir_lowering=False, debug=False,
                   enable_asserts=False)

    x_t = nc.dram_tensor("x", [n_img, N0, N0], F32, kind="ExternalInput")
    kdd_t = nc.dram_tensor("kdd", [n_chan, NP, 3 * 4 * NV], F32,
                           kind="ExternalInput")
    gt_t = nc.dram_tensor("gt", [128, 2 * NP], F32, kind="ExternalInput")
    neg67_t = nc.dram_tensor("neg67", [128, NV], F32, kind="ExternalInput")
    csth_t = nc.dram_tensor("cst_hi", [128, 384], F32, kind="ExternalInput")
    cstl_t = nc.dram_tensor("cst_lo", [4, 384], F32, kind="ExternalInput")
    rcs_t = nc.dram_tensor("rcs", [NV, 256], F32, kind="ExternalInput")
    out_t = nc.dram_tensor("out", [n_img, 2 * N0, 2 * N0], F32,
                           kind="ExternalOutput")

    PH4 = 4 * NV          # 268
    with tile.TileContext(nc) as tc:
        with (
            tc.tile_pool(name="consts", bufs=1) as cpool,
            tc.tile_pool(name="kdd", bufs=2) as kpool,
            tc.tile_pool(name="xin", bufs=3) as xpool,
            tc.tile_pool(name="r1", bufs=2) as r1pool,
            tc.tile_pool(name="ylo", bufs=2) as ylopool,
            tc.tile_pool(name="prod", bufs=2) as prodpool,
            tc.tile_pool(name="fx", bufs=2) as fxpool,
            tc.tile_pool(name="t1", bufs=2) as t1pool,
            tc.tile_pool(name="osb", bufs=2) as opool,
            tc.tile_pool(name="ppa", bufs=2, space="PSUM") as ppa_pool,
            tc.tile_pool(name="ppy", bufs=2, space="PSUM") as ppy_pool,
            tc.tile_pool(name="pt1", bufs=1, space="PSUM") as pt1_pool,
            tc.tile_pool(name="ppd", bufs=2, space="PSUM") as ppd_pool,
        ):
            gt = cpool.tile([128, 2 * NP], F32)
            nc.sync.dma_start(gt[:], gt_t[:])
            neg67 = cpool.tile([128, NV], F32)
            nc.sync.dma_start(neg67[:], neg67_t[:])
            cst_hi = cpool.tile([128, 384], F32)
            nc.sync.dma_start(cst_hi[:], csth_t[:])
            cst_lo = cpool.tile([4, 384], F32)
            nc.sync.dma_start(cst_lo[:], cstl_t[:])
            rcs = cpool.tile([NV, 256], F32)
            nc.sync.dma_start(rcs[:], rcs_t[:])

            for ci in range(n_chan):
                k_hi = kpool.tile([128, 3 * PH4], F32, tag="k_hi")
                nc.sync.dma_start(k_hi[:], kdd_t[ci, 0:128])
                k_lo = kpool.tile([4, 3 * PH4], F32, tag="k_lo")
                nc.sync.dma_start(k_lo[:], kdd_t[ci, 128:NP])

                for bi in range(n_batch):
                    img = ci * n_batch + bi

                    x_tile = xpool.tile([N0, N0], F32, tag="x")
                    nc.sync.dma_start(x_tile[:], x_t[img])

                    # ---- stage A: R1^T = x^T @ [Gr^T | Gi^T]  (PSUM) ----
                    pA = ppa_pool.tile([128, 2 * NP], F32, tag="pA")
                    nc.tensor.matmul(pA[:], x_tile[:], gt[:],
                                     start=True, stop=True)
                    r1 = r1pool.tile([128, 2 * NP], F32, tag="r1")
                    nc.scalar.activation(r1[:], pA[:], AF.Copy)

                    # ---- stage B: Y = R1 @ G^T, cols 0..66 ----
                    # pY layout: [:,0:67]=Yr_hi  [:,67:134]=Yi_hi
                    #            [0:4,134:201]=Yr_lo  [0:4,201:268]=Yi_lo
                    pY = ppy_pool.tile([128, PH4], F32, tag="pY")
                    nc.tensor.matmul(pY[:, 0:NV], r1[:, 0:128],
                                     gt[:, 0:NV], start=True, stop=False)
                    nc.tensor.matmul(pY[:, 0:NV], r1[:, NP:NP + 128],
                                     neg67[:], start=False, stop=True)
                    nc.tensor.matmul(pY[:, NV:2 * NV], r1[:, 0:128],
                                     gt[:, NP:NP + NV], start=True, stop=False)
                    nc.tensor.matmul(pY[:, NV:2 * NV], r1[:, NP:NP + 128],
                                     gt[:, 0:NV], start=False, stop=True)
                    nc.tensor.matmul(pY[0:4, 2 * NV:3 * NV], r1[:, 128:NP],
                                     gt[:, 0:NV], start=True, stop=False)
                    nc.tensor.matmul(pY[0:4, 2 * NV:3 * NV], r1[:, NP + 128:2 * NP],
                                     neg67[:], start=False, stop=True)
                    nc.tensor.matmul(pY[0:4, 3 * NV:4 * NV], r1[:, 128:NP],
                                     gt[:, NP:NP + NV], start=True, stop=False)
                    nc.tensor.matmul(pY[0:4, 3 * NV:4 * NV], r1[:, NP + 128:2 * NP],
                                     gt[:, 0:NV], start=False, stop=True)

                    # Y lo rows to SBUF for gpsimd (gpsimd cannot read PSUM)
                    ylo = ylopool.tile([4, 2 * NV], F32, tag="ylo")
                    nc.scalar.activation(ylo[:], pY[0:4, 2 * NV:4 * NV], AF.Copy)

                    # ---- FX = Kdd_hat * Y, per phase (pointwise cmul) ----
                    # hi rows on DVE, reading Y straight from PSUM
                    y_hi_b = (pY[:, 0:2 * NV]
                              .rearrange("p (a v) -> p a v", a=2)
                              [:, :, None, :]
                              .broadcast_to([128, 2, 4, NV]))
                    pa_hi = prodpool.tile([128, 2 * PH4], F32, tag="pa_hi")
                    nc.vector.tensor_mul(
                        pa_hi[:].rearrange("p (a f v) -> p a f v", a=2, f=4),
                        k_hi[:, 0:2 * PH4].rearrange("p (a f v) -> p a f v",
                                                     a=2, f=4),
                        y_hi_b)
                    pb_hi = prodpool.tile([128, 2 * PH4], F32, tag="pb_hi")
                    nc.vector.tensor_mul(
                        pb_hi[:].rearrange("p (a f v) -> p a f v", a=2, f=4),
                        k_hi[:, PH4:3 * PH4].rearrange("p (a f v) -> p a f v",
                                                       a=2, f=4),
                        y_hi_b)
                    fxr_hi = fxpool.tile([128, PH4], F32, tag="fxr_hi")
                    nc.vector.tensor_sub(fxr_hi[:], pa_hi[:, 0:PH4],
                                         pa_hi[:, PH4:2 * PH4])
                    fxi_hi = fxpool.tile([128, PH4], F32, tag="fxi_hi")
                    nc.vector.tensor_add(fxi_hi[:], pb_hi[:, 0:PH4],
                                         pb_hi[:, PH4:2 * PH4])

                    # lo rows (u=128..131) on GPSIMD
                    y_lo_b = (ylo[:]
                              .rearrange("p (a v) -> p a v", a=2)
                              [:, :, None, :]
                              .broadcast_to([4, 2, 4, NV]))
                    pa_lo = prodpool.tile([4, 2 * PH4], F32, tag="pa_lo")
                    nc.gpsimd.tensor_mul(
                        pa_lo[:].rearrange("p (a f v) -> p a f v", a=2, f=4),
                        k_lo[:, 0:2 * PH4].rearrange("p (a f v) -> p a f v",
                                                     a=2, f=4),
                        y_lo_b)
                    pb_lo = prodpool.tile([4, 2 * PH4], F32, tag="pb_lo")
                    nc.gpsimd.tensor_mul(
                        pb_lo[:].rearrange("p (a f v) -> p a f v", a=2, f=4),
                        k_lo[:, PH4:3 * PH4].rearrange("p (a f v) -> p a f v",
                                                       a=2, f=4),
                        y_lo_b)
                    fxr_lo = fxpool.tile([4, PH4], F32, tag="fxr_lo")
                    nc.gpsimd.tensor_sub(fxr_lo[:], pa_lo[:, 0:PH4],
                                         pa_lo[:, PH4:2 * PH4])
                    fxi_lo = fxpool.tile([4, PH4], F32, tag="fxi_lo")
                    nc.gpsimd.tensor_add(fxi_lo[:], pb_lo[:, 0:PH4],
                                         pb_lo[:, PH4:2 * PH4])

                    # ---- stage C': T1^T[v,x] per phase (PSUM [67,512]) ----
                    pT1r = pt1_pool.tile([NV, 512], F32, tag="pT1r")
                    pT1i = pt1_pool.tile([NV, 512], F32, tag="pT1i")
                    for p in range(4):
                        o = pT1r[:, ts(p, 128)]
                        nc.tensor.matmul(o, fxr_hi[:, ts(p, NV)],
                                         cst_hi[:, 0:128], start=True, stop=False)
                        nc.tensor.matmul(o, fxi_hi[:, ts(p, NV)],
                                         cst_hi[:, 256:384], start=False, stop=False)
                        nc.tensor.matmul(o, fxr_lo[:, ts(p, NV)],
                                         cst_lo[:, 0:128], start=False, stop=False)
                        nc.tensor.matmul(o, fxi_lo[:, ts(p, NV)],
                                         cst_lo[:, 256:384], start=False, stop=True)
                        o = pT1i[:, ts(p, 128)]
                        nc.tensor.matmul(o, fxi_hi[:, ts(p, NV)],
                                         cst_hi[:, 0:128], start=True, stop=False)
                        nc.tensor.matmul(o, fxr_hi[:, ts(p, NV)],
                                         cst_hi[:, 128:256], start=False, stop=False)
                        nc.tensor.matmul(o, fxi_lo[:, ts(p, NV)],
                                         cst_lo[:, 0:128], start=False, stop=False)
                        nc.tensor.matmul(o, fxr_lo[:, ts(p, NV)],
                                         cst_lo[:, 128:256], start=False, stop=True)

                    t1sb = t1pool.tile([NV, 1024], F32, tag="t1sb")
                    nc.scalar.activation(t1sb[:, 0:512], pT1r[:], AF.Copy)
                    nc.scalar.activation(t1sb[:, 512:1024], pT1i[:], AF.Copy)

                    # ---- stage D: out_p = T1r@RC + T1i@RS  (PSUM [128,512]) ----
                    pD = ppd_pool.tile([128, 512], F32, tag="pD")
                    for p in range(4):
                        o = pD[:, ts(p, 128)]
                        nc.tensor.matmul(o, t1sb[:, ts(p, 128)],
                                         rcs[:, 0:128], start=True, stop=False)
                        nc.tensor.matmul(o, t1sb[:, 512 + p * 128:512 + (p + 1) * 128],
                                         rcs[:, 128:256], start=False, stop=True)

                    # ---- gelu + phase interleave + store ----
                    oute = opool.tile([128, 256], F32, tag="oute")
                    outo = opool.tile([128, 256], F32, tag="outo")
                    nc.scalar.activation(
                        oute[:].rearrange("p (v d) -> p d v", d=2),
                        pD[:, 0:256].rearrange("p (d v) -> p d v", d=2),
                        act_fn)
                    nc.scalar.activation(
                        outo[:].rearrange("p (v d) -> p d v", d=2),
                        pD[:, 256:512].rearrange("p (d v) -> p d v", d=2),
                        act_fn)
                    orows = out_t[img].rearrange("(x d) y -> d x y", d=2)
                    nc.sync.dma_start(orows[0], oute[:])
                    nc.sync.dma_start(orows[1], outo[:])

    nc.compile()
    return nc


# --------------------------------------------------------------------------
# public entry point: full inputs in, full output out
# --------------------------------------------------------------------------

def kernel(x, weight, bias):
    global LAST_EXEC_NS
    x = np.ascontiguousarray(np.asarray(x, dtype=np.float32))
    consts = _host_constants(weight, bias)

    nc = build_nc()

    in_maps = []
    for core in range(NCORES):
        c0 = core * CPC
        xs = np.ascontiguousarray(
            x[:, c0:c0 + CPC].transpose(1, 0, 2, 3)).reshape(NIMG, N0, N0)
        in_maps.append({
            "x": xs,
            "kdd": np.ascontiguousarray(consts["kdd_packed"][c0:c0 + CPC]),
            "gt": consts["gt"],
            "neg67": consts["neg67"],
            "cst_hi": consts["cst_hi"],
            "cst_lo": consts["cst_lo"],
            "rcs": consts["rcs"],
        })

    trace = os.environ.get("KERNEL_TRACE", "0") == "1"
    tmpdir = os.environ.get("KERNEL_TMPDIR") or None
    res = run_bass_kernel_spmd(nc, in_maps, list(range(NCORES)), trace=trace,
                               tmpdir=tmpdir)
    LAST_EXEC_NS = res.exec_time_ns

    out = np.empty((B, C, 2 * N0, 2 * N0), dtype=np.float32)
    for core in range(NCORES):
        c0 = core * CPC
        o = res.results[core]["out"].reshape(CPC, B, 2 * N0, 2 * N0)
        out[:, c0:c0 + CPC] = o.transpose(1, 0, 2, 3)
    return out



# revision 15
# speedup vs baseline: 2.2721x; 2.2721x over previous
"""Converse2D-Up (FFT deconvolution upsampler) as a Bass/Tile kernel for TRN2.

Math (same factorization as the validated baseline, new engine mapping):
out_dd = real(IFFT132(Kdd_hat . Y)) per polyphase dd, Y = G x G^T with
G = F132 @ P (132x128).  Hermitian symmetry keeps only v=0..66 spectral
columns.  All matmuls run as float32r (TF32-class, 1 cyc/col at K=128,
N>=256), with constants or per-image tiles zero-padded so every
contraction is K=128:

  A : r1[m,u]    = x^T [Gr^T|Gi^T]                 (1 mm, N=264)
  B : Y^T[v,u]   = Gr67^T r1 + Gi67^T r1s          (2 mm, N=264)
  cmul (DVE)     : fx^T[v,(p,u)] = Kdd_hat . Y     (4 ops)
  D1': S[u,(y|y)] per phase = fx^T-slice^T [WC|WS] (8 mm, N=256)
  lo : u=128..131 handled batched per channel: 4 small mm -> S^T-lo,
       16 gathered PE transposes -> [8,(p,y)], K-padded to 128
  D2 : out[x,(p,y)] = CmT^T S_r - SmT^T S_i + lo   (3 mm, N=512)
  gelu+interleave on ScalarE eviction, row-contiguous DMA out.

Sharding: 8 channels per core x 4 batch images; weight/bias spectra are
host-precomputed constants.
"""

import os

import numpy as np

import concourse.bass as bass
import concourse.mybir as mybir
import concourse.tile as tile
from concourse import bacc
from concourse.bass import ts
from concourse.bass_utils import run_bass_kernel_spmd

F32 = mybir.dt.float32
F32R = mybir.dt.float32r
AF = mybir.ActivationFunctionType

SCALE = 2
PAD = 2
EPS = 1e-5
N0 = 128           # input spatial size
NP = N0 + 2 * PAD  # 132 padded
NU = NP * SCALE    # 264 upsampled
NV = NP // 2 + 1   # 67 unique spectral columns
B = 4
C = 64
NCORES = 8
CPC = C // NCORES  # 8 channels per core
NIMG = B * CPC     # 32 images per core

LAST_EXEC_NS = None  # set by kernel() when tracing is enabled


# --------------------------------------------------------------------------
# host-side constant precompute (weight/bias -> per-channel spectra)
# --------------------------------------------------------------------------

def _host_constants(weight, bias):
    w64 = np.asarray(weight, dtype=np.float64)
    b64 = np.asarray(bias, dtype=np.float64)

    # FB = p2o(weight): 264-point OTF of the rolled 3x3 PSF, per channel
    k_h, k_w = w64.shape[-2:]
    otf = np.zeros((C, NU, NU), dtype=np.complex128)
    otf[:, :k_h, :k_w] = w64[0]
    otf = np.roll(otf, (-(k_h // 2), -(k_w // 2)), axis=(-2, -1))
    FB = np.fft.fftn(otf, axes=(-2, -1))                      # (C,264,264)

    biaseps = 1.0 / (1.0 + np.exp(-(b64.reshape(C) - 9.0))) + EPS  # (C,)
    be = biaseps[:, None, None]

    u = np.arange(NU)
    Dr = 1 + np.exp(-2j * np.pi * u / NU)
    D = Dr[:, None] * Dr[None, :]                             # (264,264)

    Gh = np.conj(FB) + be * D[None]
    FBG = FB * Gh

    def quadmean(A):
        return 0.25 * (A[:, :NP, :NP] + A[:, NP:, :NP]
                       + A[:, :NP, NP:] + A[:, NP:, NP:])

    M1 = quadmean(FBG)
    invW = quadmean(np.abs(FB) ** 2)
    M2 = M1 / (invW + be)
    H = (Gh - np.conj(FB) * np.tile(M2, (1, SCALE, SCALE))) / be   # (C,264,264)

    hr = np.fft.ifft2(H, axes=(-2, -1)).real                  # H Hermitian
    # polyphase spectra: kdd[c,dx,dy,u,v] = FFT132(hr[c, dx::2, dy::2])[:, :NV]
    kdd = np.empty((C, 2, 2, NP, NV), dtype=np.complex128)
    for dx in range(2):
        for dy in range(2):
            kh = np.fft.fft2(hr[:, dx::2, dy::2], axes=(-2, -1))
            kdd[:, dx, dy] = kh[:, :, :NV]

    # kt packing: [c, v, (plane, p, u)] planes kt1=(kr,ki), kt2=(ki,kr)
    krT = np.ascontiguousarray(kdd.real.transpose(0, 4, 1, 2, 3)
                               ).reshape(C, NV, 4 * NP)       # (C,67,528)
    kiT = np.ascontiguousarray(kdd.imag.transpose(0, 4, 1, 2, 3)
                               ).reshape(C, NV, 4 * NP)
    kt1 = np.concatenate([krT, kiT], axis=2).astype(np.float32)  # (C,67,1056)
    kt2 = np.concatenate([kiT, krT], axis=2).astype(np.float32)

    # forward matrix G = F132 @ P  (132x128 complex)
    P = np.zeros((NP, N0))
    for m in range(NP):
        P[m, (m - PAD) % N0] = 1.0
    F132 = np.exp(-2j * np.pi * np.outer(np.arange(NP), np.arange(NP)) / NP)
    G = F132 @ P

    gtr = np.concatenate([G.real.T, G.imag.T], axis=1).astype(np.float32)  # (128,264)
    g67 = np.concatenate([G.real.T[:, :NV], G.imag.T[:, :NV]],
                         axis=1).astype(np.float32)            # (128,134)

    # inverse matrix, rows x in [2,130) of iF132/132
    Ai = np.exp(2j * np.pi * np.outer(np.arange(2, 130), np.arange(NP)) / NP) / NP
    Cm, Sm = Ai.real, Ai.imag                                  # (128,132)

    w_v = np.ones(NV)
    w_v[1:NV - 1] = 2.0
    WC = (Cm[:, :NV] * w_v[None, :]).T                         # (67,128)
    WS = (Sm[:, :NV] * w_v[None, :]).T

    def pad128(a):
        out = np.zeros((128, a.shape[1]), dtype=np.float32)
        out[:a.shape[0]] = a
        return out

    wcws = pad128(np.concatenate([WC, WS], axis=1))            # (128,256)
    nwswc = pad128(np.concatenate([-WS, WC], axis=1))          # (128,256)

    cmt = np.concatenate([Cm[:, :128].T, -Sm[:, :128].T],
                         axis=1).astype(np.float32)            # (128,256)
    cmlo = np.zeros((128, 128), dtype=np.float32)
    for j in range(4):
        cmlo[j] = Cm[:, 128 + j]          # row (ri=0, j)
        cmlo[4 + j] = -Sm[:, 128 + j]     # row (ri=1, j)

    return {
        "kt1": kt1, "kt2": kt2,
        "gtr": gtr, "g67": g67,
        "wcws": wcws, "nwswc": nwswc,
        "cmt": cmt, "cmlo": cmlo,
        "ident": np.eye(128, dtype=np.float32),
    }


# --------------------------------------------------------------------------
# device kernel
# --------------------------------------------------------------------------

def build_nc(n_chan=CPC, n_batch=B, gelu=True):
    act_fn = AF.Gelu if gelu else AF.Copy
    n_img = n_chan * n_batch
    nc = bacc.Bacc("TRN2", target_bir_lowering=False, debug=False,
                   enable_asserts=False)

    x_t = nc.dram_tensor("x", [n_img, N0, N0], F32, kind="ExternalInput")
    kt1_t = nc.dram_tensor("kt1", [n_chan, NV, 8 * NP], F32,
                           kind="ExternalInput")
    kt2_t = nc.dram_tensor("kt2", [n_chan, NV, 8 * NP], F32,
                           kind="ExternalInput")
    gtr_t = nc.dram_tensor("gtr", [128, 2 * NP], F32, kind="ExternalInput")
    g67_t = nc.dram_tensor("g67", [128, 2 * NV], F32, kind="ExternalInput")
    wcws_t = nc.dram_tensor("wcws", [128, 256], F32, kind="ExternalInput")
    nwswc_t = nc.dram_tensor("nwswc", [128, 256], F32, kind="ExternalInput")
    cmt_t = nc.dram_tensor("cmt", [128, 256], F32, kind="ExternalInput")
    cmlo_t = nc.dram_tensor("cmlo", [128, 128], F32, kind="ExternalInput")
    id_t = nc.dram_tensor("ident", [128, 128], F32, kind="ExternalInput")
    out_t = nc.dram_tensor("out", [n_img, 2 * N0, 2 * N0], F32,
                           kind="ExternalOutput")

    P4 = 4 * NP   # 528
    from contextlib import ExitStack
    with tile.TileContext(nc) as tc:
        with ExitStack() as stack:
            pool = lambda name, bufs, **kw: stack.enter_context(
                tc.tile_pool(name=name, bufs=bufs, **kw))
            cstage = pool("cstage", 1)
            cpool = pool("consts", 1)
            ktpool = pool("kt", 2)
            xpool = pool("xin", 3)
            xrpool = pool("xr", 2)
            r1pool = pool("r1", 2)
            prodpool = pool("prod", 2)
            fxpool = pool("fx", 2)
            ssbpool = pool("ssb", 2)
            slopool = pool("slo", 2)
            lokpool = pool("lok", 1)
            opool = pool("osb", 3)
            ppa_pool = pool("ppa", 1, space="PSUM")
            ppb_pool = pool("ppb", 1, space="PSUM")
            pd1_pool = pool("pd1", 1, space="PSUM")
            plo_pool = pool("plo", 1, space="PSUM")
            plt_pool = pool("plt", 2, space="PSUM")
            ppd_pool = pool("ppd", 1, space="PSUM")
            # ---- constants: DMA fp32 staging, round once into F32R ----
            def cround(t_dram, shape, tag):
                stg = cstage.tile(shape, F32, tag="stg_" + tag)
                nc.sync.dma_start(stg[:], t_dram[:])
                dst = cpool.tile(shape, F32R, tag=tag)
                nc.scalar.activation(dst[:], stg[:], AF.Copy)
                return dst

            gtr = cround(gtr_t, [128, 2 * NP], "gtr")
            g67 = cround(g67_t, [128, 2 * NV], "g67")
            wcws = cround(wcws_t, [128, 256], "wcws")
            nwswc = cround(nwswc_t, [128, 256], "nwswc")
            cmt = cround(cmt_t, [128, 256], "cmt")
            cmlo = cround(cmlo_t, [128, 128], "cmlo")
            ident = cpool.tile([128, 128], F32)
            nc.sync.dma_start(ident[:], id_t[:])

            first_fx = [True, True]   # zero pad rows once per fx buffer
            first_lok = [True] * 4
            first_lg = [True]

            for ci in range(n_chan):
                kt1 = ktpool.tile([NV, 8 * NP], F32, tag="kt1")
                nc.sync.dma_start(kt1[:], kt1_t[ci])
                kt2 = ktpool.tile([NV, 8 * NP], F32, tag="kt2")
                nc.sync.dma_start(kt2[:], kt2_t[ci])

                fxr4 = fxpool.tile([128, n_batch * P4], F32R, tag="fxr4")
                fxi4 = fxpool.tile([128, n_batch * P4], F32R, tag="fxi4")
                if first_fx[ci % 2]:
                    # rows 67:128 are K-padding read by matmuls; must not be
                    # NaN (partition base must be 32-aligned, so clear 64:128)
                    nc.vector.memset(fxr4[64:128, :].bitcast(F32), 0.0)
                    nc.vector.memset(fxi4[64:128, :].bitcast(F32), 0.0)
                    first_fx[ci % 2] = False

                pbs = []
                # ---- stage A+B per image ----
                for bi in range(n_batch):
                    img = ci * n_batch + bi
                    xf = xpool.tile([N0, N0], F32, tag="x")
                    nc.sync.dma_start(xf[:], x_t[img])
                    xr = xrpool.tile([N0, N0], F32R, tag="xr")
                    nc.gpsimd.tensor_copy(xr[:], xf[:])

                    pA = ppa_pool.tile([128, 2 * NP], F32, tag="pA")
                    nc.tensor.matmul(pA[:], xr[:], gtr[:], start=True,
                                     stop=True)

                    r1 = r1pool.tile([128, 2 * NP], F32R, tag="r1")
                    nc.scalar.activation(r1[:], pA[:], AF.Copy)
                    r1s = r1pool.tile([128, 2 * NP], F32R, tag="r1s")
                    nc.scalar.activation(r1s[:, 0:NP], pA[:, NP:2 * NP],
                                         AF.Copy, scale=-1.0)
                    nc.vector.tensor_copy(r1s[:, NP:2 * NP], pA[:, 0:NP])

                    pB = ppb_pool.tile([NV, 2 * NP], F32, tag="pB")
                    nc.tensor.matmul(pB[:], g67[:, 0:NV], r1[:],
                                     start=True, stop=False)
                    nc.tensor.matmul(pB[:], g67[:, NV:2 * NV], r1s[:],
                                     start=False, stop=True)

                    # ---- cmul: fx^T[v,(p,u)]; Y staged to SBUF so the
                    # imag half can run on GpSimd (which cannot read PSUM)
                    yb = r1pool.tile([NV, 2 * NP], F32, tag="yb")
                    nc.scalar.activation(yb[:], pB[:], AF.Copy)
                    ybc = (yb[:]
                           .rearrange("v (pl u) -> v pl u", pl=2)
                           [:, :, None, :]
                           .broadcast_to([NV, 2, 4, NP]))
                    prodA = prodpool.tile([NV, 8 * NP], F32, tag="prodA")
                    nc.vector.tensor_mul(
                        prodA[:].rearrange("v (pl p u) -> v pl p u",
                                           pl=2, p=4),
                        kt1[:].rearrange("v (pl p u) -> v pl p u",
                                         pl=2, p=4),
                        ybc)
                    nc.vector.tensor_sub(fxr4[0:NV, ts(bi, P4)],
                                         prodA[:, 0:P4], prodA[:, P4:2 * P4])
                    prodB = prodpool.tile([NV, 8 * NP], F32, tag="prodB")
                    nc.gpsimd.tensor_mul(
                        prodB[:].rearrange("v (pl p u) -> v pl p u",
                                           pl=2, p=4),
                        kt2[:].rearrange("v (pl p u) -> v pl p u",
                                         pl=2, p=4),
                        ybc)
                    nc.gpsimd.tensor_add(fxi4[0:NV, ts(bi, P4)],
                                         prodB[:, 0:P4], prodB[:, P4:2 * P4])

                # ---- D1' + Ssb eviction per image ----
                for bi in range(n_batch):
                    b0 = bi * P4
                    ps01 = pd1_pool.tile([128, 512], F32, tag="ps01")
                    ps23 = pd1_pool.tile([128, 512], F32, tag="ps23")
                    for p in range(4):
                        ps = ps01 if p < 2 else ps23
                        o = ps[:, ts(p % 2, 256)]
                        u0 = b0 + p * NP
                        nc.tensor.matmul(o, fxr4[:, u0:u0 + 128], wcws[:],
                                         start=True, stop=False)
                        nc.tensor.matmul(o, fxi4[:, u0:u0 + 128], nwswc[:],
                                         start=False, stop=True)
                    # ssb layout [u, (ri, p, y)] so D2's moving APs are flat
                    ssb = ssbpool.tile([128, 1024], F32R, tag="ssb%d" % bi)
                    sv = ssb[:].rearrange("u (ri p y) -> u p ri y",
                                          ri=2, p=4)
                    for p in range(4):
                        ps = ps01 if p < 2 else ps23
                        src = (ps[:, ts(p % 2, 256)]
                               .rearrange("u (ri y) -> u ri y", ri=2))
                        if p % 2 == 0:
                            nc.scalar.activation(sv[:, p], src, AF.Copy)
                        else:
                            nc.vector.tensor_copy(sv[:, p], src)
                    pbs.append(ssb)

                # ---- lo rows u=128..131, batched over the 4 images ----
                # stage the strided (b,p,ulo) gather into a contiguous tile
                # (matmul operand APs must be single-free-dim)
                lg = slopool.tile([128, 128], F32R, tag="logath")
                if first_lg[0]:
                    nc.vector.memset(lg[:, :].bitcast(F32), 0.0)
                    first_lg[0] = False
                nc.vector.tensor_copy(
                    lg[0:NV, 0:64].rearrange("k (b p u) -> k b p u",
                                             b=n_batch, p=4),
                    fxr4[0:NV].rearrange("k (b p u) -> k b p u",
                                         b=n_batch, p=4)[:, :, :, 128:132])
                nc.vector.tensor_copy(
                    lg[0:NV, 64:128].rearrange("k (b p u) -> k b p u",
                                               b=n_batch, p=4),
                    fxi4[0:NV].rearrange("k (b p u) -> k b p u",
                                         b=n_batch, p=4)[:, :, :, 128:132])

                plo = plo_pool.tile([128, 128], F32, tag="plo")
                nc.tensor.matmul(plo[:, 0:64], wcws[:, 0:128], lg[:, 0:64],
                                 start=True, stop=False)
                nc.tensor.matmul(plo[:, 0:64], nwswc[:, 0:128], lg[:, 64:128],
                                 start=False, stop=True)
                nc.tensor.matmul(plo[:, 64:128], wcws[:, 128:256],
                                 lg[:, 0:64], start=True, stop=False)
                nc.tensor.matmul(plo[:, 64:128], wcws[:, 0:128],
                                 lg[:, 64:128], start=False, stop=True)
                # evict interleaved to [y, (b, p, ri, u)] so each transpose
                # source is a contiguous 8-column slice
                slo = slopool.tile([128, 128], F32, tag="slo")
                nc.scalar.activation(
                    slo[:].rearrange("y (b p ri u) -> y ri b p u",
                                     ri=2, b=n_batch, p=4),
                    plo[:].rearrange("y (ri b p u) -> y ri b p u",
                                     ri=2, b=n_batch, p=4),
                    AF.Copy)

                loks = []
                for bi in range(n_batch):
                    plt = plt_pool.tile([8, 512], F32, tag="plt")
                    for p in range(4):
                        src = slo[:, bi * 32 + p * 8:bi * 32 + p * 8 + 8]
                        nc.tensor.transpose(plt[:, ts(p, 128)], src,
                                            ident[:])
                    lok = lokpool.tile([128, 512], F32R, tag="lok%d" % bi)
                    if first_lok[bi]:
                        nc.vector.memset(lok[:, :].bitcast(F32), 0.0)
                        first_lok[bi] = False
                    nc.vector.tensor_copy(lok[0:8, :], plt[:])
                    loks.append(lok)

                # ---- D2 + gelu + store per image ----
                for bi in range(n_batch):
                    img = ci * n_batch + bi
                    ssb = pbs[bi]
                    pD = ppd_pool.tile([128, 512], F32, tag="pD")
                    nc.tensor.matmul(pD[:], cmt[:, 0:128], ssb[:, 0:512],
                                     start=True, stop=False)
                    nc.tensor.matmul(pD[:], cmt[:, 128:256],
                                     ssb[:, 512:1024],
                                     start=False, stop=False)
                    nc.tensor.matmul(pD[:], cmlo[:], loks[bi][:],
                                     start=False, stop=True)

                    osb = opool.tile([128, 512], F32, tag="osb")
                    nc.scalar.activation(
                        osb[:].rearrange("x (dx y dy) -> x dx dy y",
                                         dx=2, dy=2),
                        pD[:].rearrange("x (dx dy y) -> x dx dy y",
                                        dx=2, dy=2),
                        act_fn)
                    dst = out_t[img].rearrange("(x dx) Y -> x dx Y", dx=2)
                    nc.sync.dma_start(dst, osb[:].rearrange(
                        "x (dx Y) -> x dx Y", dx=2))

    nc.compile()
    return nc


# --------------------------------------------------------------------------
# public entry point: full inputs in, full output out
# --------------------------------------------------------------------------

def kernel(x, weight, bias):
    global LAST_EXEC_NS
    x = np.ascontiguousarray(np.asarray(x, dtype=np.float32))
    consts = _host_constants(weight, bias)

    nc = build_nc()

    in_maps = []
    for core in range(NCORES):
        c0 = core * CPC
        xs = np.ascontiguousarray(
            x[:, c0:c0 + CPC].transpose(1, 0, 2, 3)).reshape(NIMG, N0, N0)
        in_maps.append({
            "x": xs,
            "kt1": np.ascontiguousarray(consts["kt1"][c0:c0 + CPC]),
            "kt2": np.ascontiguousarray(consts["kt2"][c0:c0 + CPC]),
            "gtr": consts["gtr"],
            "g67": consts["g67"],
            "wcws": consts["wcws"],
            "nwswc": consts["nwswc"],
            "cmt": consts["cmt"],
            "cmlo": consts["cmlo"],
            "ident": consts["ident"],
        })

    trace = os.environ.get("KERNEL_TRACE", "0") == "1"
    tmpdir = os.environ.get("KERNEL_TMPDIR") or None
    res = run_bass_kernel_spmd(nc, in_maps, list(range(NCORES)), trace=trace,
                               tmpdir=tmpdir)
    LAST_EXEC_NS = res.exec_time_ns

    out = np.empty((B, C, 2 * N0, 2 * N0), dtype=np.float32)
    for core in range(NCORES):
        c0 = core * CPC
        o = res.results[core]["out"].reshape(CPC, B, 2 * N0, 2 * N0)
        out[:, c0:c0 + CPC] = o.transpose(1, 0, 2, 3)
    return out


# revision 16
# speedup vs baseline: 3.4513x; 1.5190x over previous
"""Converse2D-Up (FFT deconvolution upsampler) as a Bass/Tile kernel for TRN2.

Math (same factorization as the validated baseline, new engine mapping):
out_dd = real(IFFT132(Kdd_hat . Y)) per polyphase dd, Y = G x G^T with
G = F132 @ P (132x128).  Hermitian symmetry keeps only v=0..66 spectral
columns.  All matmuls run as float32r (TF32-class, 1 cyc/col at K=128,
N>=256), with constants or per-image tiles zero-padded so every
contraction is K=128:

  A : r1[m,u]    = x^T [Gr^T|Gi^T]                 (1 mm, N=264)
  B : Y^T[v,u]   = Gr67^T r1 + Gi67^T r1s          (2 mm, N=264)
  cmul (DVE)     : fx^T[v,(p,u)] = Kdd_hat . Y     (4 ops)
  D1': S[u,(y|y)] per phase = fx^T-slice^T [WC|WS] (8 mm, N=256)
  lo : u=128..131 handled batched per channel: 4 small mm -> S^T-lo,
       16 gathered PE transposes -> [8,(p,y)], K-padded to 128
  D2 : out[x,(p,y)] = CmT^T S_r - SmT^T S_i + lo   (3 mm, N=512)
  gelu+interleave on ScalarE eviction, row-contiguous DMA out.

Sharding: 8 channels per core x 4 batch images; weight/bias spectra are
host-precomputed constants.
"""

import os

import numpy as np

import concourse.bass as bass
import concourse.mybir as mybir
import concourse.tile as tile
from concourse import bacc
from concourse.bass import ts
from concourse.bass_utils import run_bass_kernel_spmd

F32 = mybir.dt.float32
F32R = mybir.dt.float32r
BF16 = mybir.dt.bfloat16
AF = mybir.ActivationFunctionType

SCALE = 2
PAD = 2
EPS = 1e-5
N0 = 128           # input spatial size
NP = N0 + 2 * PAD  # 132 padded
NU = NP * SCALE    # 264 upsampled
NV = NP // 2 + 1   # 67 unique spectral columns
B = 4
C = 64
NCORES = 8
CPC = C // NCORES  # 8 channels per core
NIMG = B * CPC     # 32 images per core

LAST_EXEC_NS = None  # set by kernel() when tracing is enabled


# --------------------------------------------------------------------------
# host-side constant precompute (weight/bias -> per-channel spectra)
# --------------------------------------------------------------------------

def _host_constants(weight, bias):
    w64 = np.asarray(weight, dtype=np.float64)
    b64 = np.asarray(bias, dtype=np.float64)

    # FB = p2o(weight): 264-point OTF of the rolled 3x3 PSF, per channel
    k_h, k_w = w64.shape[-2:]
    otf = np.zeros((C, NU, NU), dtype=np.complex128)
    otf[:, :k_h, :k_w] = w64[0]
    otf = np.roll(otf, (-(k_h // 2), -(k_w // 2)), axis=(-2, -1))
    FB = np.fft.fftn(otf, axes=(-2, -1))                      # (C,264,264)

    biaseps = 1.0 / (1.0 + np.exp(-(b64.reshape(C) - 9.0))) + EPS  # (C,)
    be = biaseps[:, None, None]

    u = np.arange(NU)
    Dr = 1 + np.exp(-2j * np.pi * u / NU)
    D = Dr[:, None] * Dr[None, :]                             # (264,264)

    Gh = np.conj(FB) + be * D[None]
    FBG = FB * Gh

    def quadmean(A):
        return 0.25 * (A[:, :NP, :NP] + A[:, NP:, :NP]
                       + A[:, :NP, NP:] + A[:, NP:, NP:])

    M1 = quadmean(FBG)
    invW = quadmean(np.abs(FB) ** 2)
    M2 = M1 / (invW + be)
    H = (Gh - np.conj(FB) * np.tile(M2, (1, SCALE, SCALE))) / be   # (C,264,264)

    hr = np.fft.ifft2(H, axes=(-2, -1)).real                  # H Hermitian
    # polyphase spectra: kdd[c,dx,dy,u,v] = FFT132(hr[c, dx::2, dy::2])[:, :NV]
    kdd = np.empty((C, 2, 2, NP, NV), dtype=np.complex128)
    for dx in range(2):
        for dy in range(2):
            kh = np.fft.fft2(hr[:, dx::2, dy::2], axes=(-2, -1))
            kdd[:, dx, dy] = kh[:, :, :NV]

    # kt packing: [c, v, (plane, p, u)] planes kt1=(kr,ki), kt2=(ki,kr)
    krT = np.ascontiguousarray(kdd.real.transpose(0, 4, 1, 2, 3)
                               ).reshape(C, NV, 4 * NP)       # (C,67,528)
    kiT = np.ascontiguousarray(kdd.imag.transpose(0, 4, 1, 2, 3)
                               ).reshape(C, NV, 4 * NP)
    bf16 = mybir.dt.np(mybir.dt.bfloat16)
    kt1 = np.concatenate([krT, kiT], axis=2).astype(bf16)  # (C,67,1056)
    kt2 = np.concatenate([kiT, krT], axis=2).astype(bf16)

    # forward matrix G = F132 @ P  (132x128 complex)
    P = np.zeros((NP, N0))
    for m in range(NP):
        P[m, (m - PAD) % N0] = 1.0
    F132 = np.exp(-2j * np.pi * np.outer(np.arange(NP), np.arange(NP)) / NP)
    G = F132 @ P

    gtr = np.concatenate([G.real.T, G.imag.T], axis=1).astype(np.float32)  # (128,264)
    g67 = np.concatenate([G.real.T[:, :NV], G.imag.T[:, :NV]],
                         axis=1).astype(np.float32)            # (128,134)

    # inverse matrix, rows x in [2,130) of iF132/132
    Ai = np.exp(2j * np.pi * np.outer(np.arange(2, 130), np.arange(NP)) / NP) / NP
    Cm, Sm = Ai.real, Ai.imag                                  # (128,132)

    w_v = np.ones(NV)
    w_v[1:NV - 1] = 2.0
    WC = (Cm[:, :NV] * w_v[None, :]).T                         # (67,128)
    WS = (Sm[:, :NV] * w_v[None, :]).T

    def pad128(a):
        out = np.zeros((128, a.shape[1]), dtype=np.float32)
        out[:a.shape[0]] = a
        return out

    wcws = pad128(np.concatenate([WC, WS], axis=1)).astype(bf16)   # (128,256)
    nwswc = pad128(np.concatenate([-WS, WC], axis=1)).astype(bf16)

    cmt = np.concatenate([Cm[:, :128].T, -Sm[:, :128].T],
                         axis=1).astype(np.float32)            # (128,256)
    cmlo = np.zeros((128, 128), dtype=np.float32)
    for j in range(4):
        cmlo[j] = Cm[:, 128 + j]          # row (ri=0, j)
        cmlo[4 + j] = -Sm[:, 128 + j]     # row (ri=1, j)

    return {
        "kt1": kt1, "kt2": kt2,
        "gtr": gtr, "g67": g67,
        "wcws": wcws, "nwswc": nwswc,
        "cmt": cmt, "cmlo": cmlo,
        "ident": np.eye(128, dtype=np.float32),
    }


# --------------------------------------------------------------------------
# device kernel
# --------------------------------------------------------------------------

def build_nc(n_chan=CPC, n_batch=B, gelu=True):
    act_fn = AF.Gelu if gelu else AF.Copy
    n_img = n_chan * n_batch
    nc = bacc.Bacc("TRN2", target_bir_lowering=False, debug=False,
                   enable_asserts=False)

    x_t = nc.dram_tensor("x", [n_img, N0, N0], F32, kind="ExternalInput")
    kt1_t = nc.dram_tensor("kt1", [n_chan, NV, 8 * NP], BF16,
                           kind="ExternalInput")
    kt2_t = nc.dram_tensor("kt2", [n_chan, NV, 8 * NP], BF16,
                           kind="ExternalInput")
    gtr_t = nc.dram_tensor("gtr", [128, 2 * NP], F32, kind="ExternalInput")
    g67_t = nc.dram_tensor("g67", [128, 2 * NV], F32, kind="ExternalInput")
    wcws_t = nc.dram_tensor("wcws", [128, 256], BF16, kind="ExternalInput")
    nwswc_t = nc.dram_tensor("nwswc", [128, 256], BF16, kind="ExternalInput")
    cmt_t = nc.dram_tensor("cmt", [128, 256], F32, kind="ExternalInput")
    cmlo_t = nc.dram_tensor("cmlo", [128, 128], F32, kind="ExternalInput")
    id_t = nc.dram_tensor("ident", [128, 128], F32, kind="ExternalInput")
    out_t = nc.dram_tensor("out", [n_img, 2 * N0, 2 * N0], F32,
                           kind="ExternalOutput")

    P4 = 4 * NP   # 528
    from contextlib import ExitStack
    with tile.TileContext(nc) as tc:
        with ExitStack() as stack:
            pool = lambda name, bufs, **kw: stack.enter_context(
                tc.tile_pool(name=name, bufs=bufs, **kw))
            cstage = pool("cstage", 1)
            cpool = pool("consts", 1)
            ktpool = pool("kt", 2)
            xpool = pool("xin", 3)
            xrpool = pool("xr", 2)
            r1pool = pool("r1", 2)
            prodpool = pool("prod", 2)
            fxpool = pool("fx", 2)
            ssbpool = pool("ssb", 2)
            slopool = pool("slo", 2)
            lokpool = pool("lok", 1)
            opool = pool("osb", 3)
            ppa_pool = pool("ppa", 1, space="PSUM")
            ppb_pool = pool("ppb", 1, space="PSUM")
            pd1_pool = pool("pd1", 1, space="PSUM")
            plo_pool = pool("plo", 1, space="PSUM")
            plt_pool = pool("plt", 2, space="PSUM")
            ppd_pool = pool("ppd", 1, space="PSUM")
            # ---- constants: DMA fp32 staging, round once into F32R ----
            def cround(t_dram, shape, tag):
                stg = cstage.tile(shape, F32, tag="stg_" + tag)
                nc.sync.dma_start(stg[:], t_dram[:])
                dst = cpool.tile(shape, F32R, tag=tag)
                nc.scalar.activation(dst[:], stg[:], AF.Copy)
                return dst

            gtr = cround(gtr_t, [128, 2 * NP], "gtr")
            g67 = cround(g67_t, [128, 2 * NV], "g67")
            wcws = cpool.tile([128, 256], BF16, tag="wcws")
            nc.sync.dma_start(wcws[:], wcws_t[:])
            nwswc = cpool.tile([128, 256], BF16, tag="nwswc")
            nc.sync.dma_start(nwswc[:], nwswc_t[:])
            cmt = cround(cmt_t, [128, 256], "cmt")
            cmlo = cround(cmlo_t, [128, 128], "cmlo")
            ident = cpool.tile([128, 128], F32)
            nc.sync.dma_start(ident[:], id_t[:])

            first_fx = [True, True]   # zero pad rows once per fx buffer
            first_lok = [True] * 4
            first_lg = [True]

            for ci in range(n_chan):
                kt1 = ktpool.tile([NV, 8 * NP], BF16, tag="kt1")
                nc.sync.dma_start(kt1[:], kt1_t[ci])
                kt2 = ktpool.tile([NV, 8 * NP], BF16, tag="kt2")
                nc.sync.dma_start(kt2[:], kt2_t[ci])

                fxr4 = fxpool.tile([128, n_batch * P4], BF16, tag="fxr4")
                fxi4 = fxpool.tile([128, n_batch * P4], BF16, tag="fxi4")
                if first_fx[ci % 2]:
                    # rows 67:128 are K-padding read by matmuls; must not be
                    # NaN (partition base must be 32-aligned, so clear 64:128)
                    nc.vector.memset(fxr4[64:128, :], 0.0)
                    nc.vector.memset(fxi4[64:128, :], 0.0)
                    first_fx[ci % 2] = False

                pbs = []
                # ---- stage A+B per image ----
                for bi in range(n_batch):
                    img = ci * n_batch + bi
                    xf = xpool.tile([N0, N0], F32, tag="x")
                    nc.sync.dma_start(xf[:], x_t[img])
                    xr = xrpool.tile([N0, N0], F32R, tag="xr")
                    nc.gpsimd.tensor_copy(xr[:], xf[:])

                    pA = ppa_pool.tile([128, 2 * NP], F32, tag="pA")
                    nc.tensor.matmul(pA[:], xr[:], gtr[:], start=True,
                                     stop=True)

                    r1 = r1pool.tile([128, 2 * NP], F32R, tag="r1")
                    nc.scalar.activation(r1[:], pA[:], AF.Copy)
                    r1s = r1pool.tile([128, 2 * NP], F32R, tag="r1s")
                    nc.scalar.activation(r1s[:, 0:NP], pA[:, NP:2 * NP],
                                         AF.Copy, scale=-1.0)
                    nc.vector.tensor_copy(r1s[:, NP:2 * NP], pA[:, 0:NP])

                    pB = ppb_pool.tile([NV, 2 * NP], F32, tag="pB")
                    nc.tensor.matmul(pB[:], g67[:, 0:NV], r1[:],
                                     start=True, stop=False)
                    nc.tensor.matmul(pB[:], g67[:, NV:2 * NV], r1s[:],
                                     start=False, stop=True)

                    # ---- cmul: fx^T[v,(p,u)]; Y staged to SBUF so the
                    # imag half can run on GpSimd (which cannot read PSUM)
                    yb = r1pool.tile([NV, 2 * NP], BF16, tag="yb")
                    nc.scalar.activation(yb[:], pB[:], AF.Copy)
                    ybc = (yb[:]
                           .rearrange("v (pl u) -> v pl u", pl=2)
                           [:, :, None, :]
                           .broadcast_to([NV, 2, 4, NP]))
                    prodA = prodpool.tile([NV, 8 * NP], BF16, tag="prodA")
                    nc.vector.tensor_mul(
                        prodA[:].rearrange("v (pl p u) -> v pl p u",
                                           pl=2, p=4),
                        kt1[:].rearrange("v (pl p u) -> v pl p u",
                                         pl=2, p=4),
                        ybc)
                    nc.vector.tensor_sub(fxr4[0:NV, ts(bi, P4)],
                                         prodA[:, 0:P4], prodA[:, P4:2 * P4])
                    prodB = prodpool.tile([NV, 8 * NP], BF16, tag="prodB")
                    nc.gpsimd.tensor_mul(
                        prodB[:].rearrange("v (pl p u) -> v pl p u",
                                           pl=2, p=4),
                        kt2[:].rearrange("v (pl p u) -> v pl p u",
                                         pl=2, p=4),
                        ybc)
                    nc.gpsimd.tensor_add(fxi4[0:NV, ts(bi, P4)],
                                         prodB[:, 0:P4], prodB[:, P4:2 * P4])

                # ---- D1' + Ssb eviction per image ----
                for bi in range(n_batch):
                    b0 = bi * P4
                    ps01 = pd1_pool.tile([128, 512], F32, tag="ps01")
                    ps23 = pd1_pool.tile([128, 512], F32, tag="ps23")
                    for p in range(4):
                        ps = ps01 if p < 2 else ps23
                        o = ps[:, ts(p % 2, 256)]
                        u0 = b0 + p * NP
                        nc.tensor.matmul(o, fxr4[:, u0:u0 + 128], wcws[:],
                                         start=True, stop=False)
                        nc.tensor.matmul(o, fxi4[:, u0:u0 + 128], nwswc[:],
                                         start=False, stop=True)
                    # ssb layout [u, (ri, p, y)] so D2's moving APs are flat
                    ssb = ssbpool.tile([128, 1024], F32R, tag="ssb%d" % bi)
                    sv = ssb[:].rearrange("u (ri p y) -> u p ri y",
                                          ri=2, p=4)
                    for p in range(4):
                        ps = ps01 if p < 2 else ps23
                        src = (ps[:, ts(p % 2, 256)]
                               .rearrange("u (ri y) -> u ri y", ri=2))
                        if p % 2 == 0:
                            nc.scalar.activation(sv[:, p], src, AF.Copy)
                        else:
                            nc.vector.tensor_copy(sv[:, p], src)
                    pbs.append(ssb)

                # ---- lo rows u=128..131, batched over the 4 images ----
                # stage the strided (b,p,ulo) gather into a contiguous tile
                # (matmul operand APs must be single-free-dim)
                lg = slopool.tile([128, 128], BF16, tag="logath")
                if first_lg[0]:
                    nc.vector.memset(lg[:, :], 0.0)
                    first_lg[0] = False
                nc.vector.tensor_copy(
                    lg[0:NV, 0:64].rearrange("k (b p u) -> k b p u",
                                             b=n_batch, p=4),
                    fxr4[0:NV].rearrange("k (b p u) -> k b p u",
                                         b=n_batch, p=4)[:, :, :, 128:132])
                nc.vector.tensor_copy(
                    lg[0:NV, 64:128].rearrange("k (b p u) -> k b p u",
                                               b=n_batch, p=4),
                    fxi4[0:NV].rearrange("k (b p u) -> k b p u",
                                         b=n_batch, p=4)[:, :, :, 128:132])

                plo = plo_pool.tile([128, 128], F32, tag="plo")
                nc.tensor.matmul(plo[:, 0:64], wcws[:, 0:128], lg[:, 0:64],
                                 start=True, stop=False)
                nc.tensor.matmul(plo[:, 0:64], nwswc[:, 0:128], lg[:, 64:128],
                                 start=False, stop=True)
                nc.tensor.matmul(plo[:, 64:128], wcws[:, 128:256],
                                 lg[:, 0:64], start=True, stop=False)
                nc.tensor.matmul(plo[:, 64:128], wcws[:, 0:128],
                                 lg[:, 64:128], start=False, stop=True)
                # evict interleaved to [y, (b, p, ri, u)] so each transpose
                # source is a contiguous 8-column slice
                slo = slopool.tile([128, 128], F32, tag="slo")
                nc.scalar.activation(
                    slo[:].rearrange("y (b p ri u) -> y ri b p u",
                                     ri=2, b=n_batch, p=4),
                    plo[:].rearrange("y (ri b p u) -> y ri b p u",
                                     ri=2, b=n_batch, p=4),
                    AF.Copy)

                loks = []
                for bi in range(n_batch):
                    plt = plt_pool.tile([8, 512], F32, tag="plt")
                    for p in range(4):
                        src = slo[:, bi * 32 + p * 8:bi * 32 + p * 8 + 8]
                        nc.tensor.transpose(plt[:, ts(p, 128)], src,
                                            ident[:])
                    lok = lokpool.tile([128, 512], F32R, tag="lok%d" % bi)
                    if first_lok[bi]:
                        nc.vector.memset(lok[:, :].bitcast(F32), 0.0)
                        first_lok[bi] = False
                    nc.scalar.activation(lok[0:8, :], plt[:], AF.Copy)
                    loks.append(lok)

                # ---- D2 + gelu + store per image ----
                for bi in range(n_batch):
                    img = ci * n_batch + bi
                    ssb = pbs[bi]
                    pD = ppd_pool.tile([128, 512], F32, tag="pD")
                    nc.tensor.matmul(pD[:], cmt[:, 0:128], ssb[:, 0:512],
                                     start=True, stop=False)
                    nc.tensor.matmul(pD[:], cmt[:, 128:256],
                                     ssb[:, 512:1024],
                                     start=False, stop=False)
                    nc.tensor.matmul(pD[:], cmlo[:], loks[bi][:],
                                     start=False, stop=True)

                    osb = opool.tile([128, 512], F32, tag="osb")
                    nc.scalar.activation(
                        osb[:].rearrange("x (dx y dy) -> x dx dy y",
                                         dx=2, dy=2),
                        pD[:].rearrange("x (dx dy y) -> x dx dy y",
                                        dx=2, dy=2),
                        act_fn)
                    dst = out_t[img].rearrange("(x dx) Y -> x dx Y", dx=2)
                    nc.sync.dma_start(dst, osb[:].rearrange(
                        "x (dx Y) -> x dx Y", dx=2))

    nc.compile()
    return nc


# --------------------------------------------------------------------------
# public entry point: full inputs in, full output out
# --------------------------------------------------------------------------

def kernel(x, weight, bias):
    global LAST_EXEC_NS
    x = np.ascontiguousarray(np.asarray(x, dtype=np.float32))
    consts = _host_constants(weight, bias)

    nc = build_nc()

    in_maps = []
    for core in range(NCORES):
        c0 = core * CPC
        xs = np.ascontiguousarray(
            x[:, c0:c0 + CPC].transpose(1, 0, 2, 3)).reshape(NIMG, N0, N0)
        in_maps.append({
            "x": xs,
            "kt1": np.ascontiguousarray(consts["kt1"][c0:c0 + CPC]),
            "kt2": np.ascontiguousarray(consts["kt2"][c0:c0 + CPC]),
            "gtr": consts["gtr"],
            "g67": consts["g67"],
            "wcws": consts["wcws"],
            "nwswc": consts["nwswc"],
            "cmt": consts["cmt"],
            "cmlo": consts["cmlo"],
            "ident": consts["ident"],
        })

    trace = os.environ.get("KERNEL_TRACE", "0") == "1"
    tmpdir = os.environ.get("KERNEL_TMPDIR") or None
    res = run_bass_kernel_spmd(nc, in_maps, list(range(NCORES)), trace=trace,
                               tmpdir=tmpdir)
    LAST_EXEC_NS = res.exec_time_ns

    out = np.empty((B, C, 2 * N0, 2 * N0), dtype=np.float32)
    for core in range(NCORES):
        c0 = core * CPC
        o = res.results[core]["out"].reshape(CPC, B, 2 * N0, 2 * N0)
        out[:, c0:c0 + CPC] = o.transpose(1, 0, 2, 3)
    return out


# revision 18
# speedup vs baseline: 3.6217x; 1.0494x over previous
"""Converse2D-Up (FFT deconvolution upsampler) as a Bass/Tile kernel for TRN2.

Math (same factorization as the validated baseline, new engine mapping):
out_dd = real(IFFT132(Kdd_hat . Y)) per polyphase dd, Y = G x G^T with
G = F132 @ P (132x128).  Hermitian symmetry keeps only v=0..66 spectral
columns.  All matmuls run as float32r (TF32-class, 1 cyc/col at K=128,
N>=256), with constants or per-image tiles zero-padded so every
contraction is K=128:

  A : r1[m,u]    = x^T [Gr^T|Gi^T]                 (1 mm, N=264)
  B : Y^T[v,u]   = Gr67^T r1 + Gi67^T r1s          (2 mm, N=264)
  cmul (DVE)     : fx^T[v,(p,u)] = Kdd_hat . Y     (4 ops)
  D1': S[u,(y|y)] per phase = fx^T-slice^T [WC|WS] (8 mm, N=256)
  lo : u=128..131 handled batched per channel: 4 small mm -> S^T-lo,
       16 gathered PE transposes -> [8,(p,y)], K-padded to 128
  D2 : out[x,(p,y)] = CmT^T S_r - SmT^T S_i + lo   (3 mm, N=512)
  gelu+interleave on ScalarE eviction, row-contiguous DMA out.

Sharding: 8 channels per core x 4 batch images; weight/bias spectra are
host-precomputed constants.
"""

import os

import numpy as np

import concourse.bass as bass
import concourse.mybir as mybir
import concourse.tile as tile
from concourse import bacc
from concourse.bass import ts
from concourse.bass_utils import run_bass_kernel_spmd

F32 = mybir.dt.float32
F32R = mybir.dt.float32r
BF16 = mybir.dt.bfloat16
AF = mybir.ActivationFunctionType

SCALE = 2
PAD = 2
EPS = 1e-5
N0 = 128           # input spatial size
NP = N0 + 2 * PAD  # 132 padded
NU = NP * SCALE    # 264 upsampled
NV = NP // 2 + 1   # 67 unique spectral columns
B = 4
C = 64
NCORES = 8
CPC = C // NCORES  # 8 channels per core
NIMG = B * CPC     # 32 images per core

LAST_EXEC_NS = None  # set by kernel() when tracing is enabled


# --------------------------------------------------------------------------
# host-side constant precompute (weight/bias -> per-channel spectra)
# --------------------------------------------------------------------------

def _host_constants(weight, bias):
    w64 = np.asarray(weight, dtype=np.float64)
    b64 = np.asarray(bias, dtype=np.float64)

    # FB = p2o(weight): 264-point OTF of the rolled 3x3 PSF, per channel
    k_h, k_w = w64.shape[-2:]
    otf = np.zeros((C, NU, NU), dtype=np.complex128)
    otf[:, :k_h, :k_w] = w64[0]
    otf = np.roll(otf, (-(k_h // 2), -(k_w // 2)), axis=(-2, -1))
    FB = np.fft.fftn(otf, axes=(-2, -1))                      # (C,264,264)

    biaseps = 1.0 / (1.0 + np.exp(-(b64.reshape(C) - 9.0))) + EPS  # (C,)
    be = biaseps[:, None, None]

    u = np.arange(NU)
    Dr = 1 + np.exp(-2j * np.pi * u / NU)
    D = Dr[:, None] * Dr[None, :]                             # (264,264)

    Gh = np.conj(FB) + be * D[None]
    FBG = FB * Gh

    def quadmean(A):
        return 0.25 * (A[:, :NP, :NP] + A[:, NP:, :NP]
                       + A[:, :NP, NP:] + A[:, NP:, NP:])

    M1 = quadmean(FBG)
    invW = quadmean(np.abs(FB) ** 2)
    M2 = M1 / (invW + be)
    H = (Gh - np.conj(FB) * np.tile(M2, (1, SCALE, SCALE))) / be   # (C,264,264)

    hr = np.fft.ifft2(H, axes=(-2, -1)).real                  # H Hermitian
    # polyphase spectra: kdd[c,dx,dy,u,v] = FFT132(hr[c, dx::2, dy::2])[:, :NV]
    kdd = np.empty((C, 2, 2, NP, NV), dtype=np.complex128)
    for dx in range(2):
        for dy in range(2):
            kh = np.fft.fft2(hr[:, dx::2, dy::2], axes=(-2, -1))
            kdd[:, dx, dy] = kh[:, :, :NV]

    # kt packing: [c, v, (plane, p, u)] planes kt1=(kr,ki), kt2=(ki,kr)
    krT = np.ascontiguousarray(kdd.real.transpose(0, 4, 1, 2, 3)
                               ).reshape(C, NV, 4 * NP)       # (C,67,528)
    kiT = np.ascontiguousarray(kdd.imag.transpose(0, 4, 1, 2, 3)
                               ).reshape(C, NV, 4 * NP)
    bf16 = mybir.dt.np(mybir.dt.bfloat16)
    kt1 = np.concatenate([krT, kiT], axis=2).astype(bf16)  # (C,67,1056)
    kt2 = np.concatenate([kiT, krT], axis=2).astype(bf16)

    # forward matrix G = F132 @ P  (132x128 complex)
    P = np.zeros((NP, N0))
    for m in range(NP):
        P[m, (m - PAD) % N0] = 1.0
    F132 = np.exp(-2j * np.pi * np.outer(np.arange(NP), np.arange(NP)) / NP)
    G = F132 @ P

    gtr = np.concatenate([G.real.T, G.imag.T], axis=1).astype(np.float32)  # (128,264)
    g67 = np.concatenate([G.real.T[:, :NV], G.imag.T[:, :NV]],
                         axis=1).astype(np.float32)            # (128,134)

    # inverse matrix, rows x in [2,130) of iF132/132
    Ai = np.exp(2j * np.pi * np.outer(np.arange(2, 130), np.arange(NP)) / NP) / NP
    Cm, Sm = Ai.real, Ai.imag                                  # (128,132)

    w_v = np.ones(NV)
    w_v[1:NV - 1] = 2.0
    WC = (Cm[:, :NV] * w_v[None, :]).T                         # (67,128)
    WS = (Sm[:, :NV] * w_v[None, :]).T

    def pad128(a):
        out = np.zeros((128, a.shape[1]), dtype=np.float32)
        out[:a.shape[0]] = a
        return out

    wcws = pad128(np.concatenate([WC, WS], axis=1)).astype(bf16)   # (128,256)
    nwswc = pad128(np.concatenate([-WS, WC], axis=1)).astype(bf16)

    cmt = np.concatenate([Cm[:, :128].T, -Sm[:, :128].T],
                         axis=1).astype(bf16)                  # (128,256)
    cmlo = np.zeros((128, 128), dtype=np.float32)  # cast to bf16 below
    for j in range(4):
        cmlo[j] = Cm[:, 128 + j]          # row (ri=0, j)
        cmlo[4 + j] = -Sm[:, 128 + j]     # row (ri=1, j)

    return {
        "kt1": kt1, "kt2": kt2,
        "gtr": gtr, "g67": g67,
        "wcws": wcws, "nwswc": nwswc,
        "cmt": cmt, "cmlo": cmlo.astype(bf16),
        "ident": np.eye(128, dtype=np.float32),
    }


# --------------------------------------------------------------------------
# device kernel
# --------------------------------------------------------------------------

def build_nc(n_chan=CPC, n_batch=B, gelu=True):
    act_fn = AF.Gelu if gelu else AF.Copy
    n_img = n_chan * n_batch
    nc = bacc.Bacc("TRN2", target_bir_lowering=False, debug=False,
                   enable_asserts=False)

    x_t = nc.dram_tensor("x", [n_img, N0, N0], F32, kind="ExternalInput")
    kt1_t = nc.dram_tensor("kt1", [n_chan, NV, 8 * NP], BF16,
                           kind="ExternalInput")
    kt2_t = nc.dram_tensor("kt2", [n_chan, NV, 8 * NP], BF16,
                           kind="ExternalInput")
    gtr_t = nc.dram_tensor("gtr", [128, 2 * NP], F32, kind="ExternalInput")
    g67_t = nc.dram_tensor("g67", [128, 2 * NV], F32, kind="ExternalInput")
    wcws_t = nc.dram_tensor("wcws", [128, 256], BF16, kind="ExternalInput")
    nwswc_t = nc.dram_tensor("nwswc", [128, 256], BF16, kind="ExternalInput")
    cmt_t = nc.dram_tensor("cmt", [128, 256], BF16, kind="ExternalInput")
    cmlo_t = nc.dram_tensor("cmlo", [128, 128], BF16, kind="ExternalInput")
    id_t = nc.dram_tensor("ident", [128, 128], F32, kind="ExternalInput")
    out_t = nc.dram_tensor("out", [n_img, 2 * N0, 2 * N0], F32,
                           kind="ExternalOutput")

    P4 = 4 * NP   # 528
    from contextlib import ExitStack
    with tile.TileContext(nc) as tc:
        with ExitStack() as stack:
            pool = lambda name, bufs, **kw: stack.enter_context(
                tc.tile_pool(name=name, bufs=bufs, **kw))
            cstage = pool("cstage", 1)
            cpool = pool("consts", 1)
            ktpool = pool("kt", 2)
            xpool = pool("xin", 3)
            xrpool = pool("xr", 2)
            r1pool = pool("r1", 2)
            prodpool = pool("prod", 2)
            fxpool = pool("fx", 2)
            ssbpool = pool("ssb", 2)
            slopool = pool("slo", 2)
            lokpool = pool("lok", 1)
            opool = pool("osb", 3)
            ppa_pool = pool("ppa", 1, space="PSUM")
            ppb_pool = pool("ppb", 1, space="PSUM")
            pd1_pool = pool("pd1", 1, space="PSUM")
            plo_pool = pool("plo", 1, space="PSUM")
            plt_pool = pool("plt", 2, space="PSUM")
            ppd_pool = pool("ppd", 1, space="PSUM")
            # ---- constants: DMA fp32 staging, round once into F32R ----
            def cround(t_dram, shape, tag):
                stg = cstage.tile(shape, F32, tag="stg_" + tag)
                nc.sync.dma_start(stg[:], t_dram[:])
                dst = cpool.tile(shape, F32R, tag=tag)
                nc.scalar.activation(dst[:], stg[:], AF.Copy)
                return dst

            gtr = cround(gtr_t, [128, 2 * NP], "gtr")
            g67 = cround(g67_t, [128, 2 * NV], "g67")
            wcws = cpool.tile([128, 256], BF16, tag="wcws")
            nc.sync.dma_start(wcws[:], wcws_t[:])
            nwswc = cpool.tile([128, 256], BF16, tag="nwswc")
            nc.sync.dma_start(nwswc[:], nwswc_t[:])
            cmt = cpool.tile([128, 256], BF16, tag="cmt")
            nc.sync.dma_start(cmt[:], cmt_t[:])
            cmlo = cpool.tile([128, 128], BF16, tag="cmlo")
            nc.sync.dma_start(cmlo[:], cmlo_t[:])
            ident = cpool.tile([128, 128], F32)
            nc.sync.dma_start(ident[:], id_t[:])

            first_fx = [True, True]   # zero pad rows once per fx buffer
            first_lok = [True] * 4
            first_lg = [True]

            def emit_ab(ci):
                kt1 = ktpool.tile([NV, 8 * NP], BF16, tag="kt1")
                nc.sync.dma_start(kt1[:], kt1_t[ci])
                kt2 = ktpool.tile([NV, 8 * NP], BF16, tag="kt2")
                nc.sync.dma_start(kt2[:], kt2_t[ci])

                fxr4 = fxpool.tile([128, n_batch * P4], BF16, tag="fxr4")
                fxi4 = fxpool.tile([128, n_batch * P4], BF16, tag="fxi4")
                if first_fx[ci % 2]:
                    # rows 67:128 are K-padding read by matmuls; must not be
                    # NaN (partition base must be 32-aligned, so clear 64:128)
                    nc.vector.memset(fxr4[64:128, :], 0.0)
                    nc.vector.memset(fxi4[64:128, :], 0.0)
                    first_fx[ci % 2] = False

                # ---- stage A+B per image ----
                for bi in range(n_batch):
                    img = ci * n_batch + bi
                    xf = xpool.tile([N0, N0], F32, tag="x")
                    nc.sync.dma_start(xf[:], x_t[img])
                    xr = xrpool.tile([N0, N0], F32R, tag="xr")
                    nc.gpsimd.tensor_copy(xr[:], xf[:])

                    pA = ppa_pool.tile([128, 2 * NP], F32, tag="pA")
                    nc.tensor.matmul(pA[:], xr[:], gtr[:], start=True,
                                     stop=True)

                    r1 = r1pool.tile([128, 2 * NP], F32R, tag="r1")
                    nc.scalar.activation(r1[:], pA[:], AF.Copy)
                    r1s = r1pool.tile([128, 2 * NP], F32R, tag="r1s")
                    nc.scalar.activation(r1s[:, 0:NP], pA[:, NP:2 * NP],
                                         AF.Copy, scale=-1.0)
                    nc.vector.tensor_copy(r1s[:, NP:2 * NP], pA[:, 0:NP])

                    pB = ppb_pool.tile([NV, 2 * NP], F32, tag="pB")
                    nc.tensor.matmul(pB[:], g67[:, 0:NV], r1[:],
                                     start=True, stop=False)
                    nc.tensor.matmul(pB[:], g67[:, NV:2 * NV], r1s[:],
                                     start=False, stop=True)

                    # ---- cmul: fx^T[v,(p,u)]; Y staged to SBUF so the
                    # imag half can run on GpSimd (which cannot read PSUM)
                    yb = r1pool.tile([NV, 2 * NP], BF16, tag="yb")
                    nc.scalar.activation(yb[:], pB[:], AF.Copy)
                    ybc = (yb[:]
                           .rearrange("v (pl u) -> v pl u", pl=2)
                           [:, :, None, :]
                           .broadcast_to([NV, 2, 4, NP]))
                    prodA = prodpool.tile([NV, 8 * NP], BF16, tag="prodA")
                    nc.vector.tensor_mul(
                        prodA[:].rearrange("v (pl p u) -> v pl p u",
                                           pl=2, p=4),
                        kt1[:].rearrange("v (pl p u) -> v pl p u",
                                         pl=2, p=4),
                        ybc)
                    nc.vector.tensor_sub(fxr4[0:NV, ts(bi, P4)],
                                         prodA[:, 0:P4], prodA[:, P4:2 * P4])
                    prodB = prodpool.tile([NV, 8 * NP], BF16, tag="prodB")
                    nc.gpsimd.tensor_mul(
                        prodB[:].rearrange("v (pl p u) -> v pl p u",
                                           pl=2, p=4),
                        kt2[:].rearrange("v (pl p u) -> v pl p u",
                                         pl=2, p=4),
                        ybc)
                    nc.gpsimd.tensor_add(fxi4[0:NV, ts(bi, P4)],
                                         prodB[:, 0:P4], prodB[:, P4:2 * P4])
                return fxr4, fxi4

            def emit_rest(ci, fxr4, fxi4):
                pbs = []
                # ---- D1' + Ssb eviction per image ----
                for bi in range(n_batch):
                    b0 = bi * P4
                    ps01 = pd1_pool.tile([128, 512], F32, tag="ps01")
                    ps23 = pd1_pool.tile([128, 512], F32, tag="ps23")
                    for p in range(4):
                        ps = ps01 if p < 2 else ps23
                        o = ps[:, ts(p % 2, 256)]
                        u0 = b0 + p * NP
                        nc.tensor.matmul(o, fxr4[:, u0:u0 + 128], wcws[:],
                                         start=True, stop=False)
                        nc.tensor.matmul(o, fxi4[:, u0:u0 + 128], nwswc[:],
                                         start=False, stop=True)
                    # ssb layout [u, (ri, p, y)] so D2's moving APs are flat
                    ssb = ssbpool.tile([128, 1024], BF16, tag="ssb%d" % bi)
                    sv = ssb[:].rearrange("u (ri p y) -> u p ri y",
                                          ri=2, p=4)
                    for p in range(4):
                        ps = ps01 if p < 2 else ps23
                        src = (ps[:, ts(p % 2, 256)]
                               .rearrange("u (ri y) -> u ri y", ri=2))
                        if p % 2 == 0:
                            nc.scalar.activation(sv[:, p], src, AF.Copy)
                        else:
                            nc.vector.tensor_copy(sv[:, p], src)
                    pbs.append(ssb)

                # ---- lo rows u=128..131, batched over the 4 images ----
                # stage the strided (b,p,ulo) gather into a contiguous tile
                # (matmul operand APs must be single-free-dim)
                lg = slopool.tile([128, 128], BF16, tag="logath")
                if first_lg[0]:
                    nc.vector.memset(lg[:, :], 0.0)
                    first_lg[0] = False
                nc.vector.tensor_copy(
                    lg[0:NV, 0:64].rearrange("k (b p u) -> k b p u",
                                             b=n_batch, p=4),
                    fxr4[0:NV].rearrange("k (b p u) -> k b p u",
                                         b=n_batch, p=4)[:, :, :, 128:132])
                nc.vector.tensor_copy(
                    lg[0:NV, 64:128].rearrange("k (b p u) -> k b p u",
                                               b=n_batch, p=4),
                    fxi4[0:NV].rearrange("k (b p u) -> k b p u",
                                         b=n_batch, p=4)[:, :, :, 128:132])

                plo = plo_pool.tile([128, 128], F32, tag="plo")
                nc.tensor.matmul(plo[:, 0:64], wcws[:, 0:128], lg[:, 0:64],
                                 start=True, stop=False)
                nc.tensor.matmul(plo[:, 0:64], nwswc[:, 0:128], lg[:, 64:128],
                                 start=False, stop=True)
                nc.tensor.matmul(plo[:, 64:128], wcws[:, 128:256],
                                 lg[:, 0:64], start=True, stop=False)
                nc.tensor.matmul(plo[:, 64:128], wcws[:, 0:128],
                                 lg[:, 64:128], start=False, stop=True)
                # evict interleaved to [y, (b, p, ri, u)] so each transpose
                # source is a contiguous 8-column slice
                slo = slopool.tile([128, 128], F32, tag="slo")
                nc.scalar.activation(
                    slo[:].rearrange("y (b p ri u) -> y ri b p u",
                                     ri=2, b=n_batch, p=4),
                    plo[:].rearrange("y (ri b p u) -> y ri b p u",
                                     ri=2, b=n_batch, p=4),
                    AF.Copy)

                loks = []
                for bi in range(n_batch):
                    plt = plt_pool.tile([8, 512], F32, tag="plt")
                    for p in range(4):
                        src = slo[:, bi * 32 + p * 8:bi * 32 + p * 8 + 8]
                        nc.tensor.transpose(plt[:, ts(p, 128)], src,
                                            ident[:])
                    lok = lokpool.tile([128, 512], BF16, tag="lok%d" % bi)
                    if first_lok[bi]:
                        nc.vector.memset(lok[:, :], 0.0)
                        first_lok[bi] = False
                    nc.scalar.activation(lok[0:8, :], plt[:], AF.Copy)
                    loks.append(lok)

                # ---- D2 + gelu + store per image ----
                for bi in range(n_batch):
                    img = ci * n_batch + bi
                    ssb = pbs[bi]
                    pD = ppd_pool.tile([128, 512], F32, tag="pD")
                    nc.tensor.matmul(pD[:], cmt[:, 0:128], ssb[:, 0:512],
                                     start=True, stop=False)
                    nc.tensor.matmul(pD[:], cmt[:, 128:256],
                                     ssb[:, 512:1024],
                                     start=False, stop=False)
                    nc.tensor.matmul(pD[:], cmlo[:], loks[bi][:],
                                     start=False, stop=True)

                    osb = opool.tile([128, 512], F32, tag="osb")
                    nc.scalar.activation(
                        osb[:].rearrange("x (dx y dy) -> x dx dy y",
                                         dx=2, dy=2),
                        pD[:].rearrange("x (dx dy y) -> x dx dy y",
                                        dx=2, dy=2),
                        act_fn)
                    dst = out_t[img].rearrange("(x dx) Y -> x dx Y", dx=2)
                    nc.sync.dma_start(dst, osb[:].rearrange(
                        "x (dx Y) -> x dx Y", dx=2))

            # software pipeline: queue channel c+1's A/B/cmul work on the
            # engines before channel c's D1'/lo/D2 so the PE never drains
            # while a channel's cmuls finish
            chst = {}
            for ci in range(n_chan):
                chst[ci] = emit_ab(ci)
                if ci > 0:
                    emit_rest(ci - 1, *chst.pop(ci - 1))
            emit_rest(n_chan - 1, *chst.pop(n_chan - 1))

    nc.compile()
    return nc


# --------------------------------------------------------------------------
# public entry point: full inputs in, full output out
# --------------------------------------------------------------------------

def kernel(x, weight, bias):
    global LAST_EXEC_NS
    x = np.ascontiguousarray(np.asarray(x, dtype=np.float32))
    consts = _host_constants(weight, bias)

    nc = build_nc()

    in_maps = []
    for core in range(NCORES):
        c0 = core * CPC
        xs = np.ascontiguousarray(
            x[:, c0:c0 + CPC].transpose(1, 0, 2, 3)).reshape(NIMG, N0, N0)
        in_maps.append({
            "x": xs,
            "kt1": np.ascontiguousarray(consts["kt1"][c0:c0 + CPC]),
            "kt2": np.ascontiguousarray(consts["kt2"][c0:c0 + CPC]),
            "gtr": consts["gtr"],
            "g67": consts["g67"],
            "wcws": consts["wcws"],
            "nwswc": consts["nwswc"],
            "cmt": consts["cmt"],
            "cmlo": consts["cmlo"],
            "ident": consts["ident"],
        })

    trace = os.environ.get("KERNEL_TRACE", "0") == "1"
    tmpdir = os.environ.get("KERNEL_TMPDIR") or None
    res = run_bass_kernel_spmd(nc, in_maps, list(range(NCORES)), trace=trace,
                               tmpdir=tmpdir)
    LAST_EXEC_NS = res.exec_time_ns

    out = np.empty((B, C, 2 * N0, 2 * N0), dtype=np.float32)
    for core in range(NCORES):
        c0 = core * CPC
        o = res.results[core]["out"].reshape(CPC, B, 2 * N0, 2 * N0)
        out[:, c0:c0 + CPC] = o.transpose(1, 0, 2, 3)
    return out
